# revision 15
# baseline (speedup 1.0000x reference)
"""Trainium2 Bass kernel for nn_DendriticLayer.

Reference computation (all fp32 in DRAM):
    h   = leaky(x @ (Wd * dendrite_mask).T + bd)   # [B, N_SOMA]
    out = leaky(h @ (Ws * soma_mask).T + bs)       # [B, N_NEURONS]
with leaky(z) = where(z >= 0, z, 0.1 z).

Structure exploited:
  * soma_mask is block-diagonal: neuron n reads only its 16 contiguous
    dendrites (somas 16n..16n+15), so stage 2 is a tiny grouped
    contraction (done as 16 accumulating [128x128] matmuls on zero-padded
    block weights), not a dense [B,16384]x[16384,1024] matmul.
  * Sharding: somas (and their neurons) split 8 ways; core c computes
    h for somas [2048c, 2048c+2048) and out for neurons [128c, 128c+128).
    No cross-core communication.

Per-core device program, hT layout (somas on partitions, batch on free):
  wm[j]    = WdT[j] * dmaskT[j]            masked weights, bf16, resident
  for each batch block bb (512 cols of xT):
      for each soma chunk cc (128 somas):
        ph         = sum_j wm[j][:,cc].T @ xc[bb][j]     (PE, K=1024)
        hT[cc]     = Prelu(ph + bd[cc]) -> bf16          (ACT, alpha=0.1)
        pout      += wblk[cc].T @ hT[cc]                 (PE, block diag)
      out_blk = Prelu(pout + bs)                         (ACT)

This walrus build accepts only ONE semaphore wait per engine instruction,
so the kernel is written in raw Bass: every cross-engine dependency is a
standalone wait_ge on the consuming engine, with semaphore values
precomputed by a static planner.  HWDGE DMAs issued by the sync engine
complete in FIFO order, so one cumulative DMA-in semaphore suffices.

Host-side input preparation is layout-only (transpose / reshape / slice);
every arithmetic op of the reference runs on device.
"""

import numpy as np

N_CORES = 8
B = 4096
IN_DIM = 1024
N_SOMA = 16384
N_NEURONS = 1024
ND = 16                      # dendrites per neuron
P = 128
S_SH = N_SOMA // N_CORES     # 2048 somas per core
N_SH = N_NEURONS // N_CORES  # 128 neurons per core
NCH = S_SH // P              # 16 soma chunks of 128
KT = IN_DIM // P             # 8 contraction chunks of 128
BBLK = 512                   # batch block (max PE moving dim)
NB = B // BBLK               # 8 batch blocks
NQ = 4                       # weight-column quarters for the prep pipeline
WPC = S_SH // NQ             # 512 columns per weight piece
SLOPE = 0.1
NPH = 3                      # stage-1 psum buffers
NH = 4                       # hT buffers
K_TOT = NB * NCH             # 128 (bb, cc) chunks

_PROGRAM_CACHE: dict = {}


def _streams():
    """Single source of truth for all four engine instruction streams.

    Returns {engine: [op, ...]} where ops are tuples:
      ("wait", event)            wait until the event's semaphore value
      ("<op>", *args, event|None) instruction; event names its sem inc

    DMA completions on one semaphore are unordered, so a prefix-wait is
    only sound if no later DMA on that semaphore can be in flight.  DMA
    groups therefore get their own semaphores (consts / weight-quarter
    parity / x-block parity / store parity), consumers wait on whole
    groups, and issuance of the next group on a parity semaphore is gated
    on consumption of the previous one (the slot-reuse waits).
    """
    sp = []
    for name in ("bdc", "wsd", "smd", "bsc", "gmk"):
        sp.append(("dmac", name, f"ld:c:{name}"))
    for q in range(NQ):
        for j in range(KT):
            if q > 0:
                sp.append(("wait", f"mask:{q - 1}:{j}"))
            sp.append(("dmaw", "wd", q, j, f"ld:w:{q}:{j}:wd"))
            sp.append(("dmaw", "dm", q, j, f"ld:w:{q}:{j}:dm"))
        bb = q  # x group bb issued after weight quarter q
        for j in range(KT):
            if bb >= 2:
                sp.append(("wait", f"cast:{bb - 2}:{j}"))
            sp.append(("dmax", bb, j, f"ld:x:{bb}:{j}"))
    for bb in range(NQ, NB):
        for j in range(KT):
            sp.append(("wait", f"cast:{bb - 2}:{j}"))
            sp.append(("dmax", bb, j, f"ld:x:{bb}:{j}"))
        st = bb - NQ  # store bb-4 after issuing loads for bb
        sp.append(("wait", f"final:{st}"))
        sp.append(("dmao", st, f"st:{st}"))
    for st in range(NB - NQ, NB):
        sp.append(("wait", f"final:{st}"))
        sp.append(("dmao", st, f"st:{st}"))
    sp.append(("waitalldout",))

    dve = []

    def _mask_q(q):
        dve.append(("wait", f"wqdone:{q}"))
        for j in range(KT):
            dve.append(("mask", q, j, f"mask:{q}:{j}"))

    def _cast_bb(bb):
        if bb >= 2:
            dve.append(("wait", f"mm8:{16 * (bb - 2) + NCH - 1}"))
        dve.append(("wait", f"xgdone:{bb}"))
        for j in range(KT):
            dve.append(("cast", bb, j, f"cast:{bb}:{j}"))

    _mask_q(0)
    dve.append(("wait", "cdone"))
    dve.append(("wsm", "wsm"))
    dve.append(("wblkms", "wblkms"))
    # DVE is deeply pipelined: reading wsm_t/wblk back-to-back on the same
    # engine needs an explicit drain via a self-semaphore wait.
    dve.append(("wait", "wblkms"))
    for cc in range(NCH):
        dve.append(("wblk", cc, f"wblk:{cc}"))
    _cast_bb(0)
    _mask_q(1)
    _cast_bb(1)
    _mask_q(2)
    _mask_q(3)
    _cast_bb(2)
    _cast_bb(3)
    for bb in range(NQ, NB):
        _cast_bb(bb)

    act = [("wait", "cdone")]
    for k in range(K_TOT):
        bb, cc = divmod(k, NCH)
        act.append(("wait", f"mm8:{k}"))
        act.append(("evict", k, f"evict:{k}"))
        if cc == NCH - 1:
            act.append(("wait", f"s2:{k}"))
            if bb >= 2:
                act.append(("wait", f"st:{bb - 2}"))
            act.append(("final", bb, f"final:{bb}"))

    pe = []
    for k in range(K_TOT):
        bb, cc = divmod(k, NCH)
        if bb == 0:
            pe.append(("wait", f"mask:{cc // NQ}:{KT - 1}"))
            if cc == 0:
                pe.append(("wait", f"cast:0:{KT - 1}"))
                pe.append(("wait", f"wblk:{NCH - 1}"))
        elif cc == 0:
            pe.append(("wait", f"cast:{bb}:{KT - 1}"))
        if k >= NPH:
            pe.append(("wait", f"evict:{k - NPH}"))
        pe.append(("mm8", k, f"mm8:{k}"))
        if k >= 1:
            pe.append(("wait", f"evict:{k - 1}"))
            pe.append(("s2", k - 1, f"s2:{k - 1}"))
    pe.append(("wait", f"evict:{K_TOT - 1}"))
    pe.append(("s2", K_TOT - 1, f"s2:{K_TOT - 1}"))

    return {"sp": sp, "dve": dve, "act": act, "pe": pe}


def _plan_events(streams):
    """Assign each event its (sem_key, value-after-inc).

    sem_key in {c, w0, w1, x0, x1, do0, do1, dve, pe, act}.
    """
    events = {}
    counts: dict = {}

    def bump(sem, inc):
        counts[sem] = counts.get(sem, 0) + inc
        return counts[sem]

    for eng, ops in streams.items():
        for op in ops:
            kind = op[0]
            if kind in ("wait", "waitalldout"):
                continue
            ev = op[-1]
            if kind == "dmac":
                events[ev] = ("c", bump("c", 16))
            elif kind == "dmaw":
                q = op[2]
                events[ev] = (f"w{q % 2}", bump(f"w{q % 2}", 16))
            elif kind == "dmax":
                bb = op[1]
                events[ev] = (f"x{bb % 2}", bump(f"x{bb % 2}", 16))
            elif kind == "dmao":
                st = op[1]
                events[ev] = (f"do{st % 2}", bump(f"do{st % 2}", 16))
            elif eng == "dve":
                events[ev] = ("dve", bump("dve", 1))
            elif eng == "pe":
                events[ev] = ("pe", bump("pe", 1))
            elif eng == "act":
                events[ev] = ("act", bump("act", 1))
            else:
                raise ValueError((eng, kind))
    # group-done events (whole-group waits on parity semaphores)
    events["cdone"] = ("c", counts["c"])
    for q in range(NQ):
        events[f"wqdone:{q}"] = events[f"ld:w:{q}:{KT - 1}:dm"]
    for bb in range(NB):
        events[f"xgdone:{bb}"] = events[f"ld:x:{bb}:{KT - 1}"]
    events["_dout_totals"] = (counts.get("do0", 0), counts.get("do1", 0))
    return events


def build_program(mm_mode: str = "bf16", leaky_mode: str = "act"):
    import concourse.bass as bass
    import concourse.mybir as mybir

    key = (mm_mode, leaky_mode)
    if key in _PROGRAM_CACHE:
        return _PROGRAM_CACHE[key]

    f32 = mybir.dt.float32
    mm_dt = mybir.dt.bfloat16 if mm_mode == "bf16" else mybir.dt.float32r
    mult = mybir.AluOpType.mult
    prelu = mybir.ActivationFunctionType.Prelu

    nc = bass.Bass("TRN2")

    xT = nc.dram_tensor("xT", [IN_DIM, B], f32, kind="ExternalInput")
    wdT = nc.dram_tensor("wdT", [IN_DIM, S_SH], f32, kind="ExternalInput")
    dmT = nc.dram_tensor("dmT", [IN_DIM, S_SH], f32, kind="ExternalInput")
    bdc = nc.dram_tensor("bdc", [P, NCH], f32, kind="ExternalInput")
    wsd = nc.dram_tensor("wsd", [P, NCH], f32, kind="ExternalInput")
    smd = nc.dram_tensor("smd", [P, NCH], f32, kind="ExternalInput")
    bsc = nc.dram_tensor("bsc", [P, 1], f32, kind="ExternalInput")
    gmk = nc.dram_tensor("gmk", [P, 8], f32, kind="ExternalInput")
    outT = nc.dram_tensor("outT", [N_SH, B], f32, kind="ExternalOutput")
    dram_in = {"bdc": bdc, "wsd": wsd, "smd": smd, "bsc": bsc, "gmk": gmk}

    # SBUF
    wm = [nc.alloc_sbuf_tensor(f"wm{j}", [P, S_SH], mm_dt) for j in range(KT)]
    wd_st = [nc.alloc_sbuf_tensor(f"wdst{j}", [P, WPC], f32) for j in range(KT)]
    dm_st = [nc.alloc_sbuf_tensor(f"dmst{j}", [P, WPC], f32) for j in range(KT)]
    xall = [[nc.alloc_sbuf_tensor(f"xall{i}_{j}", [P, BBLK], f32)
             for j in range(KT)] for i in range(2)]
    xc = [[nc.alloc_sbuf_tensor(f"xc{i}_{j}", [P, BBLK], mm_dt)
           for j in range(KT)] for i in range(2)]
    hT = [nc.alloc_sbuf_tensor(f"hT{i}", [P, BBLK], mm_dt) for i in range(NH)]
    wblk = nc.alloc_sbuf_tensor("wblk", [P, NCH, P], mm_dt)
    osb = [nc.alloc_sbuf_tensor(f"osb{i}", [P, BBLK], f32) for i in range(2)]
    bd_t = nc.alloc_sbuf_tensor("bd_t", [P, NCH], f32)
    wsd_t = nc.alloc_sbuf_tensor("wsd_t", [P, NCH], f32)
    smd_t = nc.alloc_sbuf_tensor("smd_t", [P, NCH], f32)
    wsm_t = nc.alloc_sbuf_tensor("wsm_t", [P, NCH], f32)
    bs_t = nc.alloc_sbuf_tensor("bs_t", [P, 1], f32)
    g_t = nc.alloc_sbuf_tensor("g_t", [P, 8], f32)
    sb_in = {"bdc": bd_t, "wsd": wsd_t, "smd": smd_t, "bsc": bs_t, "gmk": g_t}

    # PSUM
    ph = [nc.alloc_psum_tensor(f"ph{i}", [P, BBLK], f32) for i in range(NPH)]
    pout = [nc.alloc_psum_tensor(f"pout{i}", [P, BBLK], f32) for i in range(2)]

    streams = _streams()
    events = _plan_events(streams)
    dout_totals = events["_dout_totals"]

    def run_stream(eng_api, ops, sems, waited):
        def wait(ev):
            sem_key, val = events[ev]
            if waited.get(sem_key, -1) >= val:
                return
            waited[sem_key] = val
            eng_api.wait_ge(sems[sem_key], val)

        def inc_of(ev):
            return sems[events[ev][0]]

        for op in ops:
            kind = op[0]
            if kind == "wait":
                wait(op[1])
            elif kind == "waitalldout":
                eng_api.wait_ge(sems["do0"], dout_totals[0])
                eng_api.wait_ge(sems["do1"], dout_totals[1])
            elif kind == "dmac":
                name, ev = op[1], op[2]
                eng_api.dma_start(sb_in[name][:], dram_in[name][:]).then_inc(
                    inc_of(ev), 16)
            elif kind == "dmaw":
                which, q, j, ev = op[1], op[2], op[3], op[4]
                dst = (wd_st if which == "wd" else dm_st)[j]
                src = (wdT if which == "wd" else dmT)
                eng_api.dma_start(
                    dst[:], src[bass.ts(j, P), bass.ts(q, WPC)]
                ).then_inc(inc_of(ev), 16)
            elif kind == "dmax":
                bb, j, ev = op[1], op[2], op[3]
                eng_api.dma_start(
                    xall[bb % 2][j][:], xT[bass.ts(j, P), bass.ts(bb, BBLK)]
                ).then_inc(inc_of(ev), 16)
            elif kind == "dmao":
                st, ev = op[1], op[2]
                eng_api.dma_start(
                    outT[:, bass.ts(st, BBLK)], osb[st % 2][:]
                ).then_inc(inc_of(ev), 16)
            elif kind == "mask":
                q, j, ev = op[1], op[2], op[3]
                nc.vector.tensor_tensor(
                    wm[j][:, bass.ts(q, WPC)], wd_st[j][:], dm_st[j][:], mult
                ).then_inc(inc_of(ev), 1)
            elif kind == "cast":
                bb, j, ev = op[1], op[2], op[3]
                nc.vector.tensor_copy(
                    xc[bb % 2][j][:], xall[bb % 2][j][:]
                ).then_inc(inc_of(ev), 1)
            elif kind == "wsm":
                nc.vector.tensor_tensor(
                    wsm_t[:], wsd_t[:], smd_t[:], mult
                ).then_inc(inc_of(op[1]), 1)
            elif kind == "wblkms":
                nc.vector.memset(wblk[:], 0.0).then_inc(inc_of(op[1]), 1)
            elif kind == "wblk":
                cc, ev = op[1], op[2]
                nc.vector.tensor_scalar_mul(
                    wblk[:, cc, 8 * cc: 8 * cc + 8], g_t[:],
                    wsm_t[:, cc: cc + 1],
                ).then_inc(inc_of(ev), 1)
            elif kind == "mm8":
                k, ev = op[1], op[2]
                bb, cc = divmod(k, NCH)
                for j in range(KT):
                    ins = nc.tensor.matmul(
                        ph[k % NPH][:],
                        wm[j][:, bass.ts(cc, P)],
                        xc[bb % 2][j][:],
                        start=(j == 0),
                        stop=(j == KT - 1),
                    )
                ins.then_inc(inc_of(ev), 1)
            elif kind == "s2":
                k, ev = op[1], op[2]
                bb, cc = divmod(k, NCH)
                nc.tensor.matmul(
                    pout[bb % 2][:],
                    wblk[:, cc, :],
                    hT[k % NH][:],
                    start=(cc == 0),
                    stop=(cc == NCH - 1),
                ).then_inc(inc_of(ev), 1)
            elif kind == "evict":
                k, ev = op[1], op[2]
                bb, cc = divmod(k, NCH)
                nc.scalar.activation(
                    hT[k % NH][:], ph[k % NPH][:], prelu,
                    bias=bd_t[:, cc: cc + 1], scale=1.0, alpha=SLOPE,
                ).then_inc(inc_of(ev), 1)
            elif kind == "final":
                bb, ev = op[1], op[2]
                nc.scalar.activation(
                    osb[bb % 2][:], pout[bb % 2][:], prelu,
                    bias=bs_t[:], scale=1.0, alpha=SLOPE,
                ).then_inc(inc_of(ev), 1)
            else:
                raise ValueError(kind)

    from contextlib import ExitStack

    with ExitStack() as es:
        sems = {
            key: es.enter_context(nc.semaphore(f"sem_{key}"))
            for key in ("c", "w0", "w1", "x0", "x1", "do0", "do1",
                        "dve", "pe", "act")
        }
        block = es.enter_context(nc.Block())

        @block.sync
        def _(sync):
            run_stream(sync, streams["sp"], sems, {})

        @block.vector
        def _(vector):
            run_stream(vector, streams["dve"], sems, {})

        @block.scalar
        def _(scalar):
            run_stream(scalar, streams["act"], sems, {})

        @block.tensor
        def _(tensor):
            run_stream(tensor, streams["pe"], sems, {})

    _PROGRAM_CACHE[key] = nc
    return nc


def make_in_maps(x, Wd, bd, Ws, bs, dendrite_mask, soma_mask):
    """Host-side sharding.  Layout-only transforms (transpose/reshape/slice):
    all reference arithmetic (masking, matmuls, bias, activations) runs on
    device."""
    f32 = np.float32
    x = np.asarray(x, f32)
    Wd = np.asarray(Wd, f32)
    bd = np.asarray(bd, f32)
    Ws = np.asarray(Ws, f32)
    bs = np.asarray(bs, f32)
    dendrite_mask = np.asarray(dendrite_mask, f32)
    soma_mask = np.asarray(soma_mask, f32)

    xT = np.ascontiguousarray(x.T)                      # [IN, B]
    WdT = np.ascontiguousarray(Wd.T)                    # [IN, N_SOMA]
    dmT = np.ascontiguousarray(dendrite_mask.T)         # [IN, N_SOMA]

    # diagonal (per-neuron) slices of the soma weights / mask
    nn_i = np.arange(N_NEURONS)[:, None]
    dd_i = ND * np.arange(N_NEURONS)[:, None] + np.arange(ND)[None, :]
    ws_diag = Ws[nn_i, dd_i]                            # [N_NEURONS, 16]
    sm_diag = soma_mask[nn_i, dd_i]                     # [N_NEURONS, 16]
    # soma_mask must be supported only on the block diagonal (it is, by
    # construction); verify cheaply so we never silently drop weight.
    assert np.count_nonzero(soma_mask) == np.count_nonzero(sm_diag), (
        "soma_mask has off-block-diagonal support; kernel sharding invalid"
    )

    wflat = ws_diag.reshape(-1)                         # [N_SOMA], soma order
    sflat = sm_diag.reshape(-1)

    gmkv = (np.arange(P)[:, None] // ND == np.arange(8)[None, :]).astype(f32)

    in_maps = []
    for c in range(N_CORES):
        sl = slice(c * S_SH, (c + 1) * S_SH)
        nl = slice(c * N_SH, (c + 1) * N_SH)
        in_maps.append(
            {
                "xT": xT,
                "wdT": np.ascontiguousarray(WdT[:, sl]),
                "dmT": np.ascontiguousarray(dmT[:, sl]),
                "bdc": np.ascontiguousarray(bd[sl].reshape(NCH, P).T),
                "wsd": np.ascontiguousarray(wflat[sl].reshape(NCH, P).T),
                "smd": np.ascontiguousarray(sflat[sl].reshape(NCH, P).T),
                "bsc": np.ascontiguousarray(bs[nl].reshape(N_SH, 1)),
                "gmk": gmkv,
            }
        )
    return in_maps


def run(inputs, trace=False, mm_mode="bf16", leaky_mode="act"):
    """Build, compile and execute on 8 NeuronCores; returns (out, results)."""
    from concourse.bass_utils import run_bass_kernel_spmd

    nc = build_program(mm_mode, leaky_mode)
    in_maps = make_in_maps(**inputs)
    res = run_bass_kernel_spmd(nc, in_maps, list(range(N_CORES)), trace=trace)
    out = np.concatenate(
        [np.asarray(res.results[c]["outT"]).T for c in range(N_CORES)], axis=1
    )
    return np.ascontiguousarray(out, dtype=np.float32), res


def kernel(**inputs) -> np.ndarray:
    return run(inputs)[0]


def bench(inputs, iters=20, warmup=3, mm_mode="bf16", leaky_mode="act"):
    """Time repeated on-device executions of the compiled program.

    Mirrors bass2jax.run_bass_via_pjrt's multi-core path, but keeps the
    jitted executable and device-resident inputs so per-iteration wall time
    = dispatch overhead + NEFF execution.  Returns (times_s, out).
    """
    import time

    import jax
    import numpy as np
    from jax.sharding import Mesh, PartitionSpec
    from jax.experimental.shard_map import shard_map

    from concourse import bass2jax
    from concourse import mybir

    bass2jax.install_neuronx_cc_hook()
    nc = build_program(mm_mode, leaky_mode)
    if not nc.is_finalized():
        nc.finalize()
    in_maps = make_in_maps(**inputs)

    partition_name = (
        nc.partition_id_tensor.name if nc.partition_id_tensor else None
    )
    in_names: list[str] = []
    out_names: list[str] = []
    out_avals = []
    zero_outs = []
    for alloc in nc.m.functions[0].allocations:
        if not isinstance(alloc, mybir.MemoryLocationSet):
            continue
        name = alloc.memorylocations[0].name
        if alloc.kind == "ExternalInput":
            if name != partition_name:
                in_names.append(name)
        elif alloc.kind == "ExternalOutput":
            out_names.append(name)
            shape = tuple(alloc.tensor_shape)
            dtype = mybir.dt.np(alloc.dtype)
            out_avals.append(jax.core.ShapedArray(shape, dtype))
            zero_outs.append(np.zeros(shape, dtype))
    n_params = len(in_names)
    all_in_names = list(in_names) + list(out_names)
    if partition_name is not None:
        all_in_names.append(partition_name)

    def _body(*args):
        operands = list(args)
        if partition_name is not None:
            operands.append(bass2jax.partition_id_tensor())
        outs = bass2jax._bass_exec_p.bind(
            *operands,
            out_avals=tuple(out_avals),
            in_names=tuple(all_in_names),
            out_names=tuple(out_names),
            lowering_input_output_aliases=(),
            sim_require_finite=True,
            sim_require_nnan=True,
            nc=nc,
        )
        return tuple(outs)

    devices = jax.devices()[:N_CORES]
    mesh = Mesh(np.asarray(devices), ("core",))
    nin = n_params + len(out_names)
    fn = jax.jit(
        shard_map(
            _body,
            mesh=mesh,
            in_specs=(PartitionSpec("core"),) * nin,
            out_specs=(PartitionSpec("core"),) * len(out_names),
            check_rep=False,
        ),
        keep_unused=True,
    )
    concat_in = [
        np.concatenate([np.asarray(in_maps[c][n]) for c in range(N_CORES)], 0)
        for n in in_names
    ]
    concat_zero = [
        np.zeros((N_CORES * z.shape[0], *z.shape[1:]), z.dtype)
        for z in zero_outs
    ]
    dev_args = [jax.device_put(a) for a in (*concat_in, *concat_zero)]
    for _ in range(warmup):
        r = fn(*dev_args)
        jax.block_until_ready(r)
    times = []
    for _ in range(iters):
        t0 = time.perf_counter()
        r = fn(*dev_args)
        jax.block_until_ready(r)
        times.append(time.perf_counter() - t0)
    outT_all = np.asarray(r[0]).reshape(N_CORES, N_SH, B)
    out = np.concatenate([outT_all[c].T for c in range(N_CORES)], axis=1)
    return times, np.ascontiguousarray(out, np.float32)


# revision 25
# speedup vs baseline: 293.4535x; 293.4535x over previous
"""Trainium2 Bass kernel for nn_DendriticLayer.

Reference computation (all fp32 in DRAM):
    h   = leaky(x @ (Wd * dendrite_mask).T + bd)   # [B, N_SOMA]
    out = leaky(h @ (Ws * soma_mask).T + bs)       # [B, N_NEURONS]
with leaky(z) = where(z >= 0, z, 0.1 z).

Structure exploited:
  * soma_mask is block-diagonal: neuron n reads only its 16 contiguous
    dendrites (somas 16n..16n+15), so stage 2 is a tiny grouped
    contraction (done as 16 accumulating [128x128] matmuls on zero-padded
    block weights), not a dense [B,16384]x[16384,1024] matmul.
  * Sharding: somas (and their neurons) split 8 ways; core c computes
    h for somas [2048c, 2048c+2048) and out for neurons [128c, 128c+128).
    No cross-core communication.

Per-core device program, hT layout (somas on partitions, batch on free):
  wm[j]    = WdT[j] * dmaskT[j]            masked weights, bf16, resident
  for each batch block bb (512 cols of xT):
      for each soma chunk cc (128 somas):
        ph         = sum_j wm[j][:,cc].T @ xc[bb][j]     (PE, K=1024)
        hT[cc]     = Prelu(ph + bd[cc]) -> bf16          (ACT, alpha=0.1)
        pout      += wblk[cc].T @ hT[cc]                 (PE, block diag)
      out_blk = Prelu(pout + bs)                         (ACT)

This walrus build accepts only ONE semaphore wait per engine instruction,
so the kernel is written in raw Bass: every cross-engine dependency is a
standalone wait_ge on the consuming engine, with semaphore values
precomputed by a static planner.  HWDGE DMAs issued by the sync engine
complete in FIFO order, so one cumulative DMA-in semaphore suffices.

Host-side input preparation is layout-only (transpose / reshape / slice);
every arithmetic op of the reference runs on device.
"""

import numpy as np

N_CORES = 8
B = 4096
IN_DIM = 1024
N_SOMA = 16384
N_NEURONS = 1024
ND = 16                      # dendrites per neuron
P = 128
S_SH = N_SOMA // N_CORES     # 2048 somas per core
N_SH = N_NEURONS // N_CORES  # 128 neurons per core
NCH = S_SH // P              # 16 soma chunks of 128
KT = IN_DIM // P             # 8 contraction chunks of 128
BBLK = 512                   # batch block (max PE moving dim)
NB = B // BBLK               # 8 batch blocks
NQ = 4                       # weight-column quarters for the prep pipeline
WPC = S_SH // NQ             # 512 columns per weight piece
SLOPE = 0.1
NPH = 3                      # stage-1 psum buffers
NH = 4                       # hT buffers
K_TOT = NB * NCH             # 128 (bb, cc) chunks

_PROGRAM_CACHE: dict = {}


def _streams(repeat: int = 1):
    """Single source of truth for all four engine instruction streams.

    Returns {engine: [op, ...]} where ops are tuples:
      ("wait", event)            wait until the event's semaphore value
      ("<op>", *args, event|None) instruction; event names its sem inc

    DMA completions on one semaphore are unordered, so a prefix-wait is
    only sound if no later DMA on that semaphore can be in flight.  DMA
    groups therefore get their own semaphores (consts / weight-quarter
    parity / x-block parity / store parity), consumers wait on whole
    groups, and issuance of the next group on a parity semaphore is gated
    on consumption of the previous one (the slot-reuse waits).
    """
    nbt = repeat * NB  # total batch blocks (same data re-processed when >NB)
    sp = []
    for name in ("bdc", "wsd", "smd", "bsc", "gmk"):
        sp.append(("dmac", name, f"ld:c:{name}"))
    for q in range(NQ):
        for j in range(KT):
            if q > 0:
                sp.append(("wait", f"mask:{q - 1}:{j}"))
            sp.append(("dmaw", "wd", q, j, f"ld:w:{q}:{j}:wd"))
            sp.append(("dmaw", "dm", q, j, f"ld:w:{q}:{j}:dm"))
        bb = q  # x group bb issued after weight quarter q
        for j in range(KT):
            if bb >= 2:
                sp.append(("wait", f"cast:{bb - 2}:{j}"))
            sp.append(("dmax", bb, j, f"ld:x:{bb}:{j}"))
    for bb in range(NQ, nbt):
        for j in range(KT):
            sp.append(("wait", f"cast:{bb - 2}:{j}"))
            sp.append(("dmax", bb, j, f"ld:x:{bb}:{j}"))
        st = bb - NQ  # store bb-4 after issuing loads for bb
        sp.append(("wait", f"final:{st}"))
        sp.append(("dmao", st, f"st:{st}"))
    for st in range(nbt - NQ, nbt):
        sp.append(("wait", f"final:{st}"))
        sp.append(("dmao", st, f"st:{st}"))
    sp.append(("waitalldout",))

    dve = []

    def _mask_q(q):
        dve.append(("wait", f"wqdone:{q}"))
        for j in range(KT):
            dve.append(("mask", q, j, f"mask:{q}:{j}"))

    def _cast_bb(bb):
        if bb >= 2:
            dve.append(("wait", f"mm8:{16 * (bb - 2) + NCH - 1}"))
        dve.append(("wait", f"xgdone:{bb}"))
        for j in range(KT):
            dve.append(("cast", bb, j, f"cast:{bb}:{j}"))

    _mask_q(0)
    dve.append(("wait", "cdone"))
    dve.append(("wsm", "wsm"))
    dve.append(("wblkms", "wblkms"))
    # DVE is deeply pipelined: reading wsm_t/wblk back-to-back on the same
    # engine needs an explicit drain via a self-semaphore wait.
    dve.append(("wait", "wblkms"))
    for cc in range(NCH):
        dve.append(("wblk", cc, f"wblk:{cc}"))
    _cast_bb(0)
    _mask_q(1)
    _cast_bb(1)
    _mask_q(2)
    _mask_q(3)
    _cast_bb(2)
    _cast_bb(3)
    for bb in range(NQ, nbt):
        _cast_bb(bb)

    k_tot = nbt * NCH
    act = [("wait", "cdone")]
    for k in range(k_tot):
        bb, cc = divmod(k, NCH)
        act.append(("wait", f"mm8:{k}"))
        act.append(("evict", k, f"evict:{k}"))
        if cc == NCH - 1:
            act.append(("wait", f"s2:{k}"))
            if bb >= 2:
                act.append(("wait", f"st:{bb - 2}"))
            act.append(("final", bb, f"final:{bb}"))

    pe = []
    for k in range(k_tot):
        bb, cc = divmod(k, NCH)
        if bb == 0:
            pe.append(("wait", f"mask:{cc // NQ}:{KT - 1}"))
            if cc == 0:
                pe.append(("wait", f"cast:0:{KT - 1}"))
                pe.append(("wait", f"wblk:{NCH - 1}"))
        elif cc == 0:
            pe.append(("wait", f"cast:{bb}:{KT - 1}"))
        if k >= NPH:
            pe.append(("wait", f"evict:{k - NPH}"))
        pe.append(("mm8", k, f"mm8:{k}"))
        if k >= 1:
            pe.append(("wait", f"evict:{k - 1}"))
            pe.append(("s2", k - 1, f"s2:{k - 1}"))
    pe.append(("wait", f"evict:{k_tot - 1}"))
    pe.append(("s2", k_tot - 1, f"s2:{k_tot - 1}"))

    return {"sp": sp, "dve": dve, "act": act, "pe": pe}


def _plan_events(streams, repeat: int = 1):
    """Assign each event its (sem_key, value-after-inc).

    sem_key in {c, w0, w1, x0, x1, do0, do1, dve, pe, act}.
    """
    events = {}
    counts: dict = {}

    def bump(sem, inc):
        counts[sem] = counts.get(sem, 0) + inc
        return counts[sem]

    for eng, ops in streams.items():
        for op in ops:
            kind = op[0]
            if kind in ("wait", "waitalldout"):
                continue
            ev = op[-1]
            if kind == "dmac":
                events[ev] = ("c", bump("c", 16))
            elif kind == "dmaw":
                q = op[2]
                events[ev] = (f"w{q % 2}", bump(f"w{q % 2}", 16))
            elif kind == "dmax":
                bb = op[1]
                events[ev] = (f"x{bb % 2}", bump(f"x{bb % 2}", 16))
            elif kind == "dmao":
                st = op[1]
                events[ev] = (f"do{st % 2}", bump(f"do{st % 2}", 16))
            elif eng == "dve":
                events[ev] = ("dve", bump("dve", 1))
            elif eng == "pe":
                events[ev] = ("pe", bump("pe", 1))
            elif eng == "act":
                events[ev] = ("act", bump("act", 1))
            else:
                raise ValueError((eng, kind))
    # group-done events (whole-group waits on parity semaphores)
    events["cdone"] = ("c", counts["c"])
    for q in range(NQ):
        events[f"wqdone:{q}"] = events[f"ld:w:{q}:{KT - 1}:dm"]
    for bb in range(repeat * NB):
        events[f"xgdone:{bb}"] = events[f"ld:x:{bb}:{KT - 1}"]
    events["_dout_totals"] = (counts.get("do0", 0), counts.get("do1", 0))
    return events


def build_program(mm_mode: str = "bf16", leaky_mode: str = "act",
                  repeat: int = 1):
    import concourse.bass as bass
    import concourse.mybir as mybir

    key = (mm_mode, leaky_mode, repeat)
    if key in _PROGRAM_CACHE:
        return _PROGRAM_CACHE[key]

    f32 = mybir.dt.float32
    mm_dt = mybir.dt.bfloat16 if mm_mode == "bf16" else mybir.dt.float32r
    mult = mybir.AluOpType.mult
    prelu = mybir.ActivationFunctionType.Prelu

    nc = bass.Bass("TRN2")

    xT = nc.dram_tensor("xT", [IN_DIM, B], f32, kind="ExternalInput")
    wdT = nc.dram_tensor("wdT", [IN_DIM, S_SH], f32, kind="ExternalInput")
    dmT = nc.dram_tensor("dmT", [IN_DIM, S_SH], f32, kind="ExternalInput")
    bdc = nc.dram_tensor("bdc", [P, NCH], f32, kind="ExternalInput")
    wsd = nc.dram_tensor("wsd", [P, NCH], f32, kind="ExternalInput")
    smd = nc.dram_tensor("smd", [P, NCH], f32, kind="ExternalInput")
    bsc = nc.dram_tensor("bsc", [P, 1], f32, kind="ExternalInput")
    gmk = nc.dram_tensor("gmk", [P, 8], f32, kind="ExternalInput")
    outT = nc.dram_tensor("outT", [N_SH, B], f32, kind="ExternalOutput")
    dram_in = {"bdc": bdc, "wsd": wsd, "smd": smd, "bsc": bsc, "gmk": gmk}

    # SBUF
    wm = [nc.alloc_sbuf_tensor(f"wm{j}", [P, S_SH], mm_dt) for j in range(KT)]
    wd_st = [nc.alloc_sbuf_tensor(f"wdst{j}", [P, WPC], f32) for j in range(KT)]
    dm_st = [nc.alloc_sbuf_tensor(f"dmst{j}", [P, WPC], f32) for j in range(KT)]
    xall = [[nc.alloc_sbuf_tensor(f"xall{i}_{j}", [P, BBLK], f32)
             for j in range(KT)] for i in range(2)]
    xc = [[nc.alloc_sbuf_tensor(f"xc{i}_{j}", [P, BBLK], mm_dt)
           for j in range(KT)] for i in range(2)]
    hT = [nc.alloc_sbuf_tensor(f"hT{i}", [P, BBLK], mm_dt) for i in range(NH)]
    wblk = nc.alloc_sbuf_tensor("wblk", [P, NCH, P], mm_dt)
    osb = [nc.alloc_sbuf_tensor(f"osb{i}", [P, BBLK], f32) for i in range(2)]
    bd_t = nc.alloc_sbuf_tensor("bd_t", [P, NCH], f32)
    wsd_t = nc.alloc_sbuf_tensor("wsd_t", [P, NCH], f32)
    smd_t = nc.alloc_sbuf_tensor("smd_t", [P, NCH], f32)
    wsm_t = nc.alloc_sbuf_tensor("wsm_t", [P, NCH], f32)
    bs_t = nc.alloc_sbuf_tensor("bs_t", [P, 1], f32)
    g_t = nc.alloc_sbuf_tensor("g_t", [P, 8], f32)
    sb_in = {"bdc": bd_t, "wsd": wsd_t, "smd": smd_t, "bsc": bs_t, "gmk": g_t}

    # PSUM
    ph = [nc.alloc_psum_tensor(f"ph{i}", [P, BBLK], f32) for i in range(NPH)]
    pout = [nc.alloc_psum_tensor(f"pout{i}", [P, BBLK], f32) for i in range(2)]

    streams = _streams(repeat)
    events = _plan_events(streams, repeat)
    dout_totals = events["_dout_totals"]

    def run_stream(eng_api, ops, sems, waited):
        def wait(ev):
            sem_key, val = events[ev]
            if waited.get(sem_key, -1) >= val:
                return
            waited[sem_key] = val
            eng_api.wait_ge(sems[sem_key], val)

        def inc_of(ev):
            return sems[events[ev][0]]

        for op in ops:
            kind = op[0]
            if kind == "wait":
                wait(op[1])
            elif kind == "waitalldout":
                eng_api.wait_ge(sems["do0"], dout_totals[0])
                eng_api.wait_ge(sems["do1"], dout_totals[1])
            elif kind == "dmac":
                name, ev = op[1], op[2]
                eng_api.dma_start(sb_in[name][:], dram_in[name][:]).then_inc(
                    inc_of(ev), 16)
            elif kind == "dmaw":
                which, q, j, ev = op[1], op[2], op[3], op[4]
                dst = (wd_st if which == "wd" else dm_st)[j]
                src = (wdT if which == "wd" else dmT)
                eng_api.dma_start(
                    dst[:], src[bass.ts(j, P), bass.ts(q, WPC)]
                ).then_inc(inc_of(ev), 16)
            elif kind == "dmax":
                bb, j, ev = op[1], op[2], op[3]
                eng_api.dma_start(
                    xall[bb % 2][j][:],
                    xT[bass.ts(j, P), bass.ts(bb % NB, BBLK)],
                ).then_inc(inc_of(ev), 16)
            elif kind == "dmao":
                st, ev = op[1], op[2]
                eng_api.dma_start(
                    outT[:, bass.ts(st % NB, BBLK)], osb[st % 2][:]
                ).then_inc(inc_of(ev), 16)
            elif kind == "mask":
                q, j, ev = op[1], op[2], op[3]
                nc.vector.tensor_tensor(
                    wm[j][:, bass.ts(q, WPC)], wd_st[j][:], dm_st[j][:], mult
                ).then_inc(inc_of(ev), 1)
            elif kind == "cast":
                bb, j, ev = op[1], op[2], op[3]
                nc.vector.tensor_copy(
                    xc[bb % 2][j][:], xall[bb % 2][j][:]
                ).then_inc(inc_of(ev), 1)
            elif kind == "wsm":
                nc.vector.tensor_tensor(
                    wsm_t[:], wsd_t[:], smd_t[:], mult
                ).then_inc(inc_of(op[1]), 1)
            elif kind == "wblkms":
                nc.vector.memset(wblk[:], 0.0).then_inc(inc_of(op[1]), 1)
            elif kind == "wblk":
                cc, ev = op[1], op[2]
                nc.vector.tensor_scalar_mul(
                    wblk[:, cc, 8 * cc: 8 * cc + 8], g_t[:],
                    wsm_t[:, cc: cc + 1],
                ).then_inc(inc_of(ev), 1)
            elif kind == "mm8":
                k, ev = op[1], op[2]
                bb, cc = divmod(k, NCH)
                for j in range(KT):
                    ins = nc.tensor.matmul(
                        ph[k % NPH][:],
                        wm[j][:, bass.ts(cc, P)],
                        xc[bb % 2][j][:],
                        start=(j == 0),
                        stop=(j == KT - 1),
                    )
                ins.then_inc(inc_of(ev), 1)
            elif kind == "s2":
                k, ev = op[1], op[2]
                bb, cc = divmod(k, NCH)
                nc.tensor.matmul(
                    pout[bb % 2][:],
                    wblk[:, cc, :],
                    hT[k % NH][:],
                    start=(cc == 0),
                    stop=(cc == NCH - 1),
                ).then_inc(inc_of(ev), 1)
            elif kind == "evict":
                k, ev = op[1], op[2]
                bb, cc = divmod(k, NCH)
                nc.scalar.activation(
                    hT[k % NH][:], ph[k % NPH][:], prelu,
                    bias=bd_t[:, cc: cc + 1], scale=1.0, alpha=SLOPE,
                ).then_inc(inc_of(ev), 1)
            elif kind == "final":
                bb, ev = op[1], op[2]
                nc.scalar.activation(
                    osb[bb % 2][:], pout[bb % 2][:], prelu,
                    bias=bs_t[:], scale=1.0, alpha=SLOPE,
                ).then_inc(inc_of(ev), 1)
            else:
                raise ValueError(kind)

    from contextlib import ExitStack

    with ExitStack() as es:
        sems = {
            key: es.enter_context(nc.semaphore(f"sem_{key}"))
            for key in ("c", "w0", "w1", "x0", "x1", "do0", "do1",
                        "dve", "pe", "act")
        }
        block = es.enter_context(nc.Block())

        @block.sync
        def _(sync):
            run_stream(sync, streams["sp"], sems, {})

        @block.vector
        def _(vector):
            run_stream(vector, streams["dve"], sems, {})

        @block.scalar
        def _(scalar):
            run_stream(scalar, streams["act"], sems, {})

        @block.tensor
        def _(tensor):
            run_stream(tensor, streams["pe"], sems, {})

    _PROGRAM_CACHE[key] = nc
    return nc


def make_in_maps(x, Wd, bd, Ws, bs, dendrite_mask, soma_mask):
    """Host-side sharding.  Layout-only transforms (transpose/reshape/slice):
    all reference arithmetic (masking, matmuls, bias, activations) runs on
    device."""
    f32 = np.float32
    x = np.asarray(x, f32)
    Wd = np.asarray(Wd, f32)
    bd = np.asarray(bd, f32)
    Ws = np.asarray(Ws, f32)
    bs = np.asarray(bs, f32)
    dendrite_mask = np.asarray(dendrite_mask, f32)
    soma_mask = np.asarray(soma_mask, f32)

    xT = np.ascontiguousarray(x.T)                      # [IN, B]
    WdT = np.ascontiguousarray(Wd.T)                    # [IN, N_SOMA]
    dmT = np.ascontiguousarray(dendrite_mask.T)         # [IN, N_SOMA]

    # diagonal (per-neuron) slices of the soma weights / mask
    nn_i = np.arange(N_NEURONS)[:, None]
    dd_i = ND * np.arange(N_NEURONS)[:, None] + np.arange(ND)[None, :]
    ws_diag = Ws[nn_i, dd_i]                            # [N_NEURONS, 16]
    sm_diag = soma_mask[nn_i, dd_i]                     # [N_NEURONS, 16]
    # soma_mask must be supported only on the block diagonal (it is, by
    # construction); verify cheaply so we never silently drop weight.
    assert np.count_nonzero(soma_mask) == np.count_nonzero(sm_diag), (
        "soma_mask has off-block-diagonal support; kernel sharding invalid"
    )

    wflat = ws_diag.reshape(-1)                         # [N_SOMA], soma order
    sflat = sm_diag.reshape(-1)

    gmkv = (np.arange(P)[:, None] // ND == np.arange(8)[None, :]).astype(f32)

    in_maps = []
    for c in range(N_CORES):
        sl = slice(c * S_SH, (c + 1) * S_SH)
        nl = slice(c * N_SH, (c + 1) * N_SH)
        in_maps.append(
            {
                "xT": xT,
                "wdT": np.ascontiguousarray(WdT[:, sl]),
                "dmT": np.ascontiguousarray(dmT[:, sl]),
                "bdc": np.ascontiguousarray(bd[sl].reshape(NCH, P).T),
                "wsd": np.ascontiguousarray(wflat[sl].reshape(NCH, P).T),
                "smd": np.ascontiguousarray(sflat[sl].reshape(NCH, P).T),
                "bsc": np.ascontiguousarray(bs[nl].reshape(N_SH, 1)),
                "gmk": gmkv,
            }
        )
    return in_maps


def run(inputs, trace=False, mm_mode="bf16", leaky_mode="act"):
    """Build, compile and execute on 8 NeuronCores; returns (out, results)."""
    from concourse.bass_utils import run_bass_kernel_spmd

    nc = build_program(mm_mode, leaky_mode)
    in_maps = make_in_maps(**inputs)
    res = run_bass_kernel_spmd(nc, in_maps, list(range(N_CORES)), trace=trace)
    out = np.concatenate(
        [np.asarray(res.results[c]["outT"]).T for c in range(N_CORES)], axis=1
    )
    return np.ascontiguousarray(out, dtype=np.float32), res


def kernel(**inputs) -> np.ndarray:
    return run(inputs)[0]


def bench(inputs, iters=20, warmup=3, mm_mode="bf16", leaky_mode="act",
          repeat=1):
    """Time repeated on-device executions of the compiled program.

    Mirrors bass2jax.run_bass_via_pjrt's multi-core path, but keeps the
    jitted executable and device-resident inputs so per-iteration wall time
    = dispatch overhead + NEFF execution.  Returns (times_s, out).
    """
    import time

    import jax
    import numpy as np
    from jax.sharding import Mesh, PartitionSpec
    from jax.experimental.shard_map import shard_map

    from concourse import bass2jax
    from concourse import mybir

    bass2jax.install_neuronx_cc_hook()
    nc = build_program(mm_mode, leaky_mode, repeat)
    if not nc.is_finalized():
        nc.finalize()
    in_maps = make_in_maps(**inputs)

    partition_name = (
        nc.partition_id_tensor.name if nc.partition_id_tensor else None
    )
    in_names: list[str] = []
    out_names: list[str] = []
    out_avals = []
    zero_outs = []
    for alloc in nc.m.functions[0].allocations:
        if not isinstance(alloc, mybir.MemoryLocationSet):
            continue
        name = alloc.memorylocations[0].name
        if alloc.kind == "ExternalInput":
            if name != partition_name:
                in_names.append(name)
        elif alloc.kind == "ExternalOutput":
            out_names.append(name)
            shape = tuple(alloc.tensor_shape)
            dtype = mybir.dt.np(alloc.dtype)
            out_avals.append(jax.core.ShapedArray(shape, dtype))
            zero_outs.append(np.zeros(shape, dtype))
    n_params = len(in_names)
    all_in_names = list(in_names) + list(out_names)
    if partition_name is not None:
        all_in_names.append(partition_name)

    def _body(*args):
        operands = list(args)
        if partition_name is not None:
            operands.append(bass2jax.partition_id_tensor())
        outs = bass2jax._bass_exec_p.bind(
            *operands,
            out_avals=tuple(out_avals),
            in_names=tuple(all_in_names),
            out_names=tuple(out_names),
            lowering_input_output_aliases=(),
            sim_require_finite=True,
            sim_require_nnan=True,
            nc=nc,
        )
        return tuple(outs)

    devices = jax.devices()[:N_CORES]
    mesh = Mesh(np.asarray(devices), ("core",))
    nin = n_params + len(out_names)
    fn = jax.jit(
        shard_map(
            _body,
            mesh=mesh,
            in_specs=(PartitionSpec("core"),) * nin,
            out_specs=(PartitionSpec("core"),) * len(out_names),
            check_rep=False,
        ),
        keep_unused=True,
    )
    concat_in = [
        np.concatenate([np.asarray(in_maps[c][n]) for c in range(N_CORES)], 0)
        for n in in_names
    ]
    concat_zero = [
        np.zeros((N_CORES * z.shape[0], *z.shape[1:]), z.dtype)
        for z in zero_outs
    ]
    dev_args = [jax.device_put(a) for a in (*concat_in, *concat_zero)]
    for _ in range(warmup):
        r = fn(*dev_args)
        jax.block_until_ready(r)
    times = []
    for _ in range(iters):
        t0 = time.perf_counter()
        r = fn(*dev_args)
        jax.block_until_ready(r)
        times.append(time.perf_counter() - t0)
    outT_all = np.asarray(r[0]).reshape(N_CORES, N_SH, B)
    out = np.concatenate([outT_all[c].T for c in range(N_CORES)], axis=1)
    return times, np.ascontiguousarray(out, np.float32)


# revision 79
# speedup vs baseline: 338.8403x; 1.1547x over previous
"""Trainium2 Bass kernel for nn_DendriticLayer.

Reference computation (all fp32 in DRAM):
    h   = leaky(x @ (Wd * dendrite_mask).T + bd)   # [B, N_SOMA]
    out = leaky(h @ (Ws * soma_mask).T + bs)       # [B, N_NEURONS]
with leaky(z) = where(z >= 0, z, 0.1 z).

Structure exploited:
  * soma_mask is block-diagonal: neuron n reads only its 16 contiguous
    dendrites (somas 16n..16n+15), so stage 2 is a tiny grouped
    contraction (done as 16 accumulating [128x128] matmuls on zero-padded
    block weights), not a dense [B,16384]x[16384,1024] matmul.
  * Sharding: somas (and their neurons) split 8 ways; core c computes
    h for somas [2048c, 2048c+2048) and out for neurons [128c, 128c+128).
    No cross-core communication.

Per-core device program, hT layout (somas on partitions, batch on free):
  wm[j]    = WdT[j] * dmaskT[j]            masked weights, bf16, resident
  for each batch block bb (512 cols of xT):
      for each soma chunk cc (128 somas):
        ph         = sum_j wm[j][:,cc].T @ xc[bb][j]     (PE, K=1024)
        hT[cc]     = Prelu(ph + bd[cc]) -> bf16          (ACT, alpha=0.1)
        pout      += wblk[cc].T @ hT[cc]                 (PE, block diag)
      out_blk = Prelu(pout + bs)                         (ACT)

This walrus build accepts only ONE semaphore wait per engine instruction,
so the kernel is written in raw Bass: every cross-engine dependency is a
standalone wait_ge on the consuming engine, with semaphore values
precomputed by a static planner.  HWDGE DMAs issued by the sync engine
complete in FIFO order, so one cumulative DMA-in semaphore suffices.

Host-side input preparation is layout-only (transpose / reshape / slice);
every arithmetic op of the reference runs on device.
"""

import numpy as np

N_CORES = 8
B = 4096
IN_DIM = 1024
N_SOMA = 16384
N_NEURONS = 1024
ND = 16                      # dendrites per neuron
P = 128
S_SH = N_SOMA // N_CORES     # 2048 somas per core
N_SH = N_NEURONS // N_CORES  # 128 neurons per core
NCH = S_SH // P              # 16 soma chunks of 128
KT = IN_DIM // P             # 8 contraction chunks of 128
BBLK = 512                   # batch block (max PE moving dim)
NB = B // BBLK               # 8 batch blocks
NQ = 4                       # weight-column quarters for the prep pipeline
WPC = S_SH // NQ             # 512 columns per weight piece
SLOPE = 0.1
NPH = 4                      # stage-1 psum buffers
NH = 6                       # hT buffers
K_TOT = NB * NCH             # 128 (bb, cc) chunks

_PROGRAM_CACHE: dict = {}


def _streams(repeat: int = 1, variant: str = "full"):
    """Single source of truth for all four engine instruction streams.

    Returns {engine: [op, ...]} where ops are tuples:
      ("wait", event)            wait until the event's semaphore value
      ("<op>", *args, event|None) instruction; event names its sem inc

    DMA completions on one semaphore are unordered, so a prefix-wait is
    only sound if no later DMA on that semaphore can be in flight.  DMA
    groups therefore get their own semaphores (consts / weight-quarter
    parity / x-block parity / store parity), consumers wait on whole
    groups, and issuance of the next group on a parity semaphore is gated
    on consumption of the previous one (the slot-reuse waits).
    """
    nbt = repeat * NB  # total batch blocks (same data re-processed when >NB)
    sp = []
    for name in ("bdc", "wsd", "smd", "bsc", "gmk"):
        sp.append(("dmac", name, f"ld:c:{name}"))
    for q in range(NQ):
        for j in range(KT):
            if q > 0:
                sp.append(("wait", f"mask:{q - 1}:{j}"))
            sp.append(("dmaw", "wd", q, j, f"ld:w:{q}:{j}:wd"))
            sp.append(("dmaw", "dm", q, j, f"ld:w:{q}:{j}:dm"))
        bb = q  # x group bb issued after weight quarter q
        for j in range(KT):
            if bb >= 2:
                sp.append(("wait", f"cast:{bb - 2}:{j}"))
            sp.append(("dmax", bb, j, f"ld:x:{bb}:{j}"))
    for bb in range(NQ, nbt):
        for j in range(KT):
            sp.append(("wait", f"cast:{bb - 2}:{j}"))
            sp.append(("dmax", bb, j, f"ld:x:{bb}:{j}"))
        st = bb - NQ  # store bb-4 after issuing loads for bb
        sp.append(("wait", f"final:{st}"))
        sp.append(("dmao", st, f"st:{st}"))
    for st in range(nbt - NQ, nbt):
        sp.append(("wait", f"final:{st}"))
        sp.append(("dmao", st, f"st:{st}"))
    sp.append(("waitalldout",))

    dve = []

    def _mask_q(q):
        dve.append(("wait", f"wqdone:{q}"))
        for j in range(KT):
            dve.append(("mask", q, j, f"mask:{q}:{j}"))

    def _cast_bb(bb):
        if bb >= 2:
            dve.append(("wait", f"mm8:{16 * (bb - 2) + NCH - 1}"))
        dve.append(("wait", f"xgdone:{bb}"))
        for j in range(KT):
            dve.append(("cast", bb, j, f"cast:{bb}:{j}"))

    _mask_q(0)
    dve.append(("wait", "cdone"))
    dve.append(("wsm", "wsm"))
    dve.append(("wblkms", "wblkms"))
    # DVE is deeply pipelined: reading wsm_t/wblk back-to-back on the same
    # engine needs an explicit drain via a self-semaphore wait.
    dve.append(("wait", "wblkms"))
    for cc in range(NCH):
        dve.append(("wblk", cc, f"wblk:{cc}"))
    _cast_bb(0)
    _mask_q(1)
    _cast_bb(1)
    _mask_q(2)
    _mask_q(3)
    _cast_bb(2)
    _cast_bb(3)
    for bb in range(NQ, nbt):
        _cast_bb(bb)

    k_tot = nbt * NCH
    no_s2 = variant == "nos2"
    act = [("wait", "cdone")]
    for k in range(k_tot):
        bb, cc = divmod(k, NCH)
        act.append(("wait", f"mm8:{k}"))
        act.append(("evict", k, f"evict:{k}"))
        if cc == NCH - 1:
            if not no_s2:
                act.append(("wait", f"s2:{k}"))
            if bb >= 2:
                act.append(("wait", f"st:{bb - 2}"))
            act.append(("final", bb, f"final:{bb}"))

    pe = []
    s2_next = 0  # next stage-2 chunk to emit (they stay in order)

    def _flush_s2(upto):
        # emit pending stage-2 matmuls for chunks < upto; one wait covers
        # the batch since ACT evicts in order
        nonlocal s2_next
        if no_s2 or upto <= s2_next:
            return
        if s2_next == 0:
            pe.append(("wait", f"wblk:{NCH - 1}"))
        pe.append(("wait", f"evict:{upto - 1}"))
        while s2_next < upto:
            pe.append(("s2", s2_next, f"s2:{s2_next}"))
            s2_next += 1

    for k in range(k_tot):
        bb, cc = divmod(k, NCH)
        if bb == 0:
            pe.append(("wait", f"mask:{cc // NQ}:{KT - 1}"))
            if cc == 0:
                pe.append(("wait", f"cast:0:{KT - 1}"))
        elif cc == 0:
            pe.append(("wait", f"cast:{bb}:{KT - 1}"))
        if k >= NPH:
            pe.append(("wait", f"evict:{k - NPH}"))
        pe.append(("mm8", k, f"mm8:{k}"))
        # stage-2 runs 2 chunks behind, in pairs (fewer waits, more slack
        # for the ACT eviction to land).  NOTE: the flush wait evict(k-2)
        # must stay behind ACT's final(bb) block point (which waits on the
        # last s2 of bb) or the engines deadlock; pairs satisfy this.
        if k % 2 == 1:
            _flush_s2(k - 1)
    _flush_s2(k_tot)

    return {"sp": sp, "dve": dve, "act": act, "pe": pe}


def _plan_events(streams, repeat: int = 1):
    """Assign each event its (sem_key, value-after-inc).

    sem_key in {c, w0, w1, x0, x1, do0, do1, dve, pe, act}.
    """
    events = {}
    counts: dict = {}

    def bump(sem, inc):
        counts[sem] = counts.get(sem, 0) + inc
        return counts[sem]

    for eng, ops in streams.items():
        for op in ops:
            kind = op[0]
            if kind in ("wait", "waitalldout"):
                continue
            ev = op[-1]
            if kind == "dmac":
                events[ev] = ("c", bump("c", 16))
            elif kind == "dmaw":
                q = op[2]
                events[ev] = (f"w{q % 2}", bump(f"w{q % 2}", 16))
            elif kind == "dmax":
                bb = op[1]
                events[ev] = (f"x{bb % 2}", bump(f"x{bb % 2}", 16))
            elif kind == "dmao":
                st = op[1]
                events[ev] = (f"do{st % 2}", bump(f"do{st % 2}", 16))
            elif eng == "dve":
                events[ev] = ("dve", bump("dve", 1))
            elif eng == "pe":
                events[ev] = ("pe", bump("pe", 1))
            elif eng == "act":
                events[ev] = ("act", bump("act", 1))
            else:
                raise ValueError((eng, kind))
    # group-done events (whole-group waits on parity semaphores)
    events["cdone"] = ("c", counts["c"])
    for q in range(NQ):
        events[f"wqdone:{q}"] = events[f"ld:w:{q}:{KT - 1}:dm"]
    for bb in range(repeat * NB):
        events[f"xgdone:{bb}"] = events[f"ld:x:{bb}:{KT - 1}"]
    events["_dout_totals"] = (counts.get("do0", 0), counts.get("do1", 0))
    return events


def build_program(mm_mode: str = "bf16", leaky_mode: str = "act",
                  repeat: int = 1, variant: str = "full"):
    import concourse.bass as bass
    import concourse.mybir as mybir

    key = (mm_mode, leaky_mode, repeat, variant)
    if key in _PROGRAM_CACHE:
        return _PROGRAM_CACHE[key]

    f32 = mybir.dt.float32
    mm_dt = mybir.dt.bfloat16 if mm_mode == "bf16" else mybir.dt.float32r
    mult = mybir.AluOpType.mult
    prelu = mybir.ActivationFunctionType.Prelu

    nc = bass.Bass("TRN2")

    bf16 = mybir.dt.bfloat16
    xT = nc.dram_tensor("xT", [IN_DIM, B], f32, kind="ExternalInput")
    # Weights/mask ship as bf16: the mask is exactly 0/1 in bf16, so
    # bf16(Wd)*mask == bf16(Wd*mask) bit-for-bit while halving the cold
    # weight-DMA prefix that gates the first batch block.
    wdT = nc.dram_tensor("wdT", [IN_DIM, S_SH], bf16, kind="ExternalInput")
    dmT = nc.dram_tensor("dmT", [IN_DIM, S_SH], bf16, kind="ExternalInput")
    bdc = nc.dram_tensor("bdc", [P, NCH], f32, kind="ExternalInput")
    wsd = nc.dram_tensor("wsd", [P, NCH], f32, kind="ExternalInput")
    smd = nc.dram_tensor("smd", [P, NCH], f32, kind="ExternalInput")
    bsc = nc.dram_tensor("bsc", [P, 1], f32, kind="ExternalInput")
    gmk = nc.dram_tensor("gmk", [P, 8], f32, kind="ExternalInput")
    outT = nc.dram_tensor("outT", [N_SH, B], f32, kind="ExternalOutput")
    dram_in = {"bdc": bdc, "wsd": wsd, "smd": smd, "bsc": bsc, "gmk": gmk}

    # SBUF
    wm = [nc.alloc_sbuf_tensor(f"wm{j}", [P, S_SH], mm_dt) for j in range(KT)]
    wd_st = [nc.alloc_sbuf_tensor(f"wdst{j}", [P, WPC], bf16) for j in range(KT)]
    dm_st = [nc.alloc_sbuf_tensor(f"dmst{j}", [P, WPC], bf16) for j in range(KT)]
    xall = [[nc.alloc_sbuf_tensor(f"xall{i}_{j}", [P, BBLK], f32)
             for j in range(KT)] for i in range(2)]
    xc = [[nc.alloc_sbuf_tensor(f"xc{i}_{j}", [P, BBLK], mm_dt)
           for j in range(KT)] for i in range(2)]
    hT = [nc.alloc_sbuf_tensor(f"hT{i}", [P, BBLK], mm_dt) for i in range(NH)]
    wblk = nc.alloc_sbuf_tensor("wblk", [P, NCH, P], mm_dt)
    osb = [nc.alloc_sbuf_tensor(f"osb{i}", [P, BBLK], f32) for i in range(2)]
    bd_t = nc.alloc_sbuf_tensor("bd_t", [P, NCH], f32)
    wsd_t = nc.alloc_sbuf_tensor("wsd_t", [P, NCH], f32)
    smd_t = nc.alloc_sbuf_tensor("smd_t", [P, NCH], f32)
    wsm_t = nc.alloc_sbuf_tensor("wsm_t", [P, NCH], f32)
    bs_t = nc.alloc_sbuf_tensor("bs_t", [P, 1], f32)
    g_t = nc.alloc_sbuf_tensor("g_t", [P, 8], f32)
    sb_in = {"bdc": bd_t, "wsd": wsd_t, "smd": smd_t, "bsc": bs_t, "gmk": g_t}

    # PSUM
    ph = [nc.alloc_psum_tensor(f"ph{i}", [P, BBLK], f32) for i in range(NPH)]
    pout = [nc.alloc_psum_tensor(f"pout{i}", [P, BBLK], f32) for i in range(2)]
    ps_spare = (nc.alloc_psum_tensor("ps_spare", [P, BBLK], f32)
                if variant == "s2dup" else None)

    streams = _streams(repeat, variant)
    events = _plan_events(streams, repeat)
    dout_totals = events["_dout_totals"]

    def run_stream(eng_api, ops, sems, waited):
        def wait(ev):
            sem_key, val = events[ev]
            if waited.get(sem_key, -1) >= val:
                return
            waited[sem_key] = val
            eng_api.wait_ge(sems[sem_key], val)

        def inc_of(ev):
            return sems[events[ev][0]]

        for op in ops:
            kind = op[0]
            if kind == "wait":
                wait(op[1])
            elif kind == "waitalldout":
                eng_api.wait_ge(sems["do0"], dout_totals[0])
                eng_api.wait_ge(sems["do1"], dout_totals[1])
            elif kind == "dmac":
                name, ev = op[1], op[2]
                eng_api.dma_start(sb_in[name][:], dram_in[name][:]).then_inc(
                    inc_of(ev), 16)
            elif kind == "dmaw":
                which, q, j, ev = op[1], op[2], op[3], op[4]
                dst = (wd_st if which == "wd" else dm_st)[j]
                src = (wdT if which == "wd" else dmT)
                eng_api.dma_start(
                    dst[:], src[bass.ts(j, P), bass.ts(q, WPC)]
                ).then_inc(inc_of(ev), 16)
            elif kind == "dmax":
                bb, j, ev = op[1], op[2], op[3]
                eng_api.dma_start(
                    xall[bb % 2][j][:],
                    xT[bass.ts(j, P), bass.ts(bb % NB, BBLK)],
                ).then_inc(inc_of(ev), 16)
            elif kind == "dmao":
                st, ev = op[1], op[2]
                eng_api.dma_start(
                    outT[:, bass.ts(st % NB, BBLK)], osb[st % 2][:]
                ).then_inc(inc_of(ev), 16)
            elif kind == "mask":
                q, j, ev = op[1], op[2], op[3]
                nc.vector.tensor_tensor(
                    wm[j][:, bass.ts(q, WPC)], wd_st[j][:], dm_st[j][:], mult
                ).then_inc(inc_of(ev), 1)
            elif kind == "cast":
                bb, j, ev = op[1], op[2], op[3]
                nc.vector.tensor_copy(
                    xc[bb % 2][j][:], xall[bb % 2][j][:]
                ).then_inc(inc_of(ev), 1)
            elif kind == "wsm":
                nc.vector.tensor_tensor(
                    wsm_t[:], wsd_t[:], smd_t[:], mult
                ).then_inc(inc_of(op[1]), 1)
            elif kind == "wblkms":
                nc.vector.memset(wblk[:], 0.0).then_inc(inc_of(op[1]), 1)
            elif kind == "wblk":
                cc, ev = op[1], op[2]
                nc.vector.tensor_scalar_mul(
                    wblk[:, cc, 8 * cc: 8 * cc + 8], g_t[:],
                    wsm_t[:, cc: cc + 1],
                ).then_inc(inc_of(ev), 1)
            elif kind == "mm8":
                k, ev = op[1], op[2]
                bb, cc = divmod(k, NCH)
                for j in range(KT):
                    ins = nc.tensor.matmul(
                        ph[k % NPH][:],
                        wm[j][:, bass.ts(cc, P)],
                        xc[bb % 2][j][:],
                        start=(j == 0),
                        stop=(j == KT - 1),
                    )
                ins.then_inc(inc_of(ev), 1)
            elif kind == "s2":
                k, ev = op[1], op[2]
                bb, cc = divmod(k, NCH)
                if variant == "s2dup":
                    # timing probe: same shapes, fresh start/stop group in a
                    # bank nothing reads (output is garbage)
                    nc.tensor.matmul(
                        ps_spare[:], wblk[:, cc, :], hT[k % NH][:],
                        start=True, stop=True,
                    ).then_inc(inc_of(ev), 1)
                else:
                    nc.tensor.matmul(
                        pout[bb % 2][:],
                        wblk[:, cc, :],
                        hT[k % NH][:],
                        start=(cc == 0),
                        stop=(cc == NCH - 1),
                    ).then_inc(inc_of(ev), 1)
            elif kind == "evict":
                k, ev = op[1], op[2]
                bb, cc = divmod(k, NCH)
                nc.scalar.activation(
                    hT[k % NH][:], ph[k % NPH][:], prelu,
                    bias=bd_t[:, cc: cc + 1], scale=1.0, alpha=SLOPE,
                ).then_inc(inc_of(ev), 1)
            elif kind == "final":
                bb, ev = op[1], op[2]
                src = hT[0] if variant in ("nos2", "s2dup") else pout[bb % 2]
                nc.scalar.activation(
                    osb[bb % 2][:], src[:], prelu,
                    bias=bs_t[:], scale=1.0, alpha=SLOPE,
                ).then_inc(inc_of(ev), 1)
            else:
                raise ValueError(kind)

    from contextlib import ExitStack

    with ExitStack() as es:
        sems = {
            key: es.enter_context(nc.semaphore(f"sem_{key}"))
            for key in ("c", "w0", "w1", "x0", "x1", "do0", "do1",
                        "dve", "pe", "act")
        }
        block = es.enter_context(nc.Block())

        @block.sync
        def _(sync):
            run_stream(sync, streams["sp"], sems, {})

        @block.vector
        def _(vector):
            run_stream(vector, streams["dve"], sems, {})

        @block.scalar
        def _(scalar):
            run_stream(scalar, streams["act"], sems, {})

        @block.tensor
        def _(tensor):
            run_stream(tensor, streams["pe"], sems, {})

    _PROGRAM_CACHE[key] = nc
    return nc


def make_in_maps(x, Wd, bd, Ws, bs, dendrite_mask, soma_mask):
    """Host-side sharding.  Layout-only transforms (transpose/reshape/slice):
    all reference arithmetic (masking, matmuls, bias, activations) runs on
    device."""
    f32 = np.float32
    x = np.asarray(x, f32)
    Wd = np.asarray(Wd, f32)
    bd = np.asarray(bd, f32)
    Ws = np.asarray(Ws, f32)
    bs = np.asarray(bs, f32)
    dendrite_mask = np.asarray(dendrite_mask, f32)
    soma_mask = np.asarray(soma_mask, f32)

    import ml_dtypes

    bf16 = ml_dtypes.bfloat16
    xT = np.ascontiguousarray(x.T)                      # [IN, B]
    # bf16 shipping is numerically exact w.r.t. the device-side bf16
    # pipeline: dendrite_mask is 0/1 (exact in bf16) and the masking
    # multiply still runs on device.
    WdT = np.ascontiguousarray(Wd.T.astype(bf16))       # [IN, N_SOMA]
    dmT = np.ascontiguousarray(dendrite_mask.T.astype(bf16))

    # diagonal (per-neuron) slices of the soma weights / mask
    nn_i = np.arange(N_NEURONS)[:, None]
    dd_i = ND * np.arange(N_NEURONS)[:, None] + np.arange(ND)[None, :]
    ws_diag = Ws[nn_i, dd_i]                            # [N_NEURONS, 16]
    sm_diag = soma_mask[nn_i, dd_i]                     # [N_NEURONS, 16]
    # soma_mask must be supported only on the block diagonal (it is, by
    # construction); verify cheaply so we never silently drop weight.
    assert np.count_nonzero(soma_mask) == np.count_nonzero(sm_diag), (
        "soma_mask has off-block-diagonal support; kernel sharding invalid"
    )

    wflat = ws_diag.reshape(-1)                         # [N_SOMA], soma order
    sflat = sm_diag.reshape(-1)

    gmkv = (np.arange(P)[:, None] // ND == np.arange(8)[None, :]).astype(f32)

    in_maps = []
    for c in range(N_CORES):
        sl = slice(c * S_SH, (c + 1) * S_SH)
        nl = slice(c * N_SH, (c + 1) * N_SH)
        in_maps.append(
            {
                "xT": xT,
                "wdT": np.ascontiguousarray(WdT[:, sl]),
                "dmT": np.ascontiguousarray(dmT[:, sl]),
                "bdc": np.ascontiguousarray(bd[sl].reshape(NCH, P).T),
                "wsd": np.ascontiguousarray(wflat[sl].reshape(NCH, P).T),
                "smd": np.ascontiguousarray(sflat[sl].reshape(NCH, P).T),
                "bsc": np.ascontiguousarray(bs[nl].reshape(N_SH, 1)),
                "gmk": gmkv,
            }
        )
    return in_maps


def run(inputs, trace=False, mm_mode="bf16", leaky_mode="act"):
    """Build, compile and execute on 8 NeuronCores; returns (out, results)."""
    from concourse.bass_utils import run_bass_kernel_spmd

    nc = build_program(mm_mode, leaky_mode)
    in_maps = make_in_maps(**inputs)
    res = run_bass_kernel_spmd(nc, in_maps, list(range(N_CORES)), trace=trace)
    out = np.concatenate(
        [np.asarray(res.results[c]["outT"]).T for c in range(N_CORES)], axis=1
    )
    return np.ascontiguousarray(out, dtype=np.float32), res


def kernel(**inputs) -> np.ndarray:
    return run(inputs)[0]


def bench(inputs, iters=20, warmup=3, mm_mode="bf16", leaky_mode="act",
          repeat=1, variant="full"):
    """Time repeated on-device executions of the compiled program.

    Mirrors bass2jax.run_bass_via_pjrt's multi-core path, but keeps the
    jitted executable and device-resident inputs so per-iteration wall time
    = dispatch overhead + NEFF execution.  Returns (times_s, out).
    """
    import time

    import jax
    import numpy as np
    from jax.sharding import Mesh, PartitionSpec
    from jax.experimental.shard_map import shard_map

    from concourse import bass2jax
    from concourse import mybir

    bass2jax.install_neuronx_cc_hook()
    nc = build_program(mm_mode, leaky_mode, repeat, variant)
    if not nc.is_finalized():
        nc.finalize()
    in_maps = make_in_maps(**inputs)

    partition_name = (
        nc.partition_id_tensor.name if nc.partition_id_tensor else None
    )
    in_names: list[str] = []
    out_names: list[str] = []
    out_avals = []
    zero_outs = []
    for alloc in nc.m.functions[0].allocations:
        if not isinstance(alloc, mybir.MemoryLocationSet):
            continue
        name = alloc.memorylocations[0].name
        if alloc.kind == "ExternalInput":
            if name != partition_name:
                in_names.append(name)
        elif alloc.kind == "ExternalOutput":
            out_names.append(name)
            shape = tuple(alloc.tensor_shape)
            dtype = mybir.dt.np(alloc.dtype)
            out_avals.append(jax.core.ShapedArray(shape, dtype))
            zero_outs.append(np.zeros(shape, dtype))
    n_params = len(in_names)
    all_in_names = list(in_names) + list(out_names)
    if partition_name is not None:
        all_in_names.append(partition_name)

    def _body(*args):
        operands = list(args)
        if partition_name is not None:
            operands.append(bass2jax.partition_id_tensor())
        outs = bass2jax._bass_exec_p.bind(
            *operands,
            out_avals=tuple(out_avals),
            in_names=tuple(all_in_names),
            out_names=tuple(out_names),
            lowering_input_output_aliases=(),
            sim_require_finite=True,
            sim_require_nnan=True,
            nc=nc,
        )
        return tuple(outs)

    devices = jax.devices()[:N_CORES]
    mesh = Mesh(np.asarray(devices), ("core",))
    nin = n_params + len(out_names)
    fn = jax.jit(
        shard_map(
            _body,
            mesh=mesh,
            in_specs=(PartitionSpec("core"),) * nin,
            out_specs=(PartitionSpec("core"),) * len(out_names),
            check_rep=False,
        ),
        keep_unused=True,
    )
    concat_in = [
        np.concatenate([np.asarray(in_maps[c][n]) for c in range(N_CORES)], 0)
        for n in in_names
    ]
    concat_zero = [
        np.zeros((N_CORES * z.shape[0], *z.shape[1:]), z.dtype)
        for z in zero_outs
    ]
    dev_args = [jax.device_put(a) for a in (*concat_in, *concat_zero)]
    for _ in range(warmup):
        r = fn(*dev_args)
        jax.block_until_ready(r)
    times = []
    for _ in range(iters):
        t0 = time.perf_counter()
        r = fn(*dev_args)
        jax.block_until_ready(r)
        times.append(time.perf_counter() - t0)
    outT_all = np.asarray(r[0]).reshape(N_CORES, N_SH, B)
    out = np.concatenate([outT_all[c].T for c in range(N_CORES)], axis=1)
    return times, np.ascontiguousarray(out, np.float32)
